# revision 15
# baseline (speedup 1.0000x reference)
"""Trainium2 Bass kernel for a custom LSTM cell step.

Reference computation (per full problem, B=8192, D=U=512):
    z = inputs @ kernel + h_tm1 @ recurrent_kernel + bias        # [B, 4U]
    i, f, g, o = split(z, 4, axis=1)
    i, f, o = sigmoid(...)  ;  g = tanh(g)
    c = f * c_tm1 + i * g
    h = o * tanh(c)
    return (h, h, c)

Sharding: data-parallel over the batch dim across 8 NeuronCores
(1024 rows per core); weights replicated.

Kernel strategy (final, ~72.7us vs 91.9us f32r baseline):
  - The PE is the bottleneck. Measured on hw: one [K=128]x[128,512]
    matmul streams at ~216ns regardless of f32r/bf16 (1 cyc/row at
    ~2.4GHz; DVFS sometimes caps it at ~259ns). fp8 DoubleRow doubles
    K per instruction but the 3-term hi/lo decomposition needed for
    precision (pure fp8 rel err 3.8e-2 > 2e-2 gate) costs 1.5x bf16's
    cycles — measured 117us — so bf16 is optimal: 256 matmuls ~= 55us.
  - x/h are cast to bf16 AND pre-transposed on the host into a stacked
    [x^T; h^T] tensor: no PE transposes, no PSUM->SBUF copies. W/R are
    host-stacked [W; R] in bf16 (halves all weight DMA).
    bf16 quantization end-to-end rel err ~2.4e-3 (gate is 2e-2).
  - DMA: concurrently-active queues contend for the ~400GB/s fabric
    and delay each other's completions, so ALL inputs go FIFO down the
    single sync queue in exact first-use order (xh/Wi single-kg slabs
    interleaved, then Wf, c, Wg+Wo); transfers keep 1-2KB contiguous
    runs. Five junk matmuls on a zeroed tile warm the PE out of its
    low-p-state while the first slabs land (~9us).
  - i-phase runs kg-outer/mt-inner across all 8 PSUM banks, paced by
    the arriving xh slabs; the f-phase (mt-outer) also precomputes
    fc = f*c_old; the fused g+o phase finishes each m-tile completely
    and streams c/h out immediately (outputs cap at ~131GB/s/queue, so
    they must overlap the matmul stream; h rides the scalar queue and
    gpsimd stays DMA-free for a short teardown drain).
  - The last m-tile runs in half-width chunks striped across the
    sync/scalar queues, halving the serial sigmoid->tanh->mul->DMA
    tail after the final matmul. Measured tail ~6us incl. ~2.8us fixed
    NEFF teardown; fixed preamble before the first DMA is ~7us.
"""

from contextlib import ExitStack

import ml_dtypes
import numpy as np

import concourse.bass as bass
import concourse.mybir as mybir
import concourse.tile as tile
from concourse import bacc
from concourse.bass_utils import run_bass_kernel_spmd

# Problem sizes (hardcoded per spec).
B, D, U = 8192, 512, 512
N_CORES = 8
MB = B // N_CORES  # 1024 batch rows per core
P = 128
MT = MB // P  # 8 m-tiles per core
KK = D + U  # 1024 stacked contraction dim (x|h vs W|R)
KG = KK // P  # 8 k-groups of 128
NG = 4 * U  # 2048 gate columns
NC = 512  # gate chunk width (one gate)

F32 = mybir.dt.float32
BF16 = mybir.dt.bfloat16
BF16NP = ml_dtypes.bfloat16

SIG = mybir.ActivationFunctionType.Sigmoid
TANH = mybir.ActivationFunctionType.Tanh

# Gate column chunks: 0=i, 1=f, 2=g, 3=o
GI, GF, GG, GO = 0, 1, 2, 3

_NC_CACHE: dict = {}


def _build_lstm_nc(with_bias: bool):
    """Build and compile the per-core Bass program."""
    nc = bacc.Bacc("TRN2", target_bir_lowering=False, debug=False)

    xh_d = nc.dram_tensor("xh_t", [KK, MB], BF16, kind="ExternalInput")
    wr_d = nc.dram_tensor("wr", [KK, NG], BF16, kind="ExternalInput")
    c_d = nc.dram_tensor("c_tm1", [MB, U], F32, kind="ExternalInput")
    b_d = None
    if with_bias:
        b_d = nc.dram_tensor("bias", [NG], F32, kind="ExternalInput")
    ho_d = nc.dram_tensor("h_out", [MB, U], F32, kind="ExternalOutput")
    co_d = nc.dram_tensor("c_out", [MB, U], F32, kind="ExternalOutput")

    # DRAM views tiled to [partition, group, free]
    xh_v = xh_d.ap().rearrange("(kg p) m -> p kg m", p=P)
    wr_v = wr_d.ap().rearrange("(kg p) n -> p kg n", p=P)
    c_v = c_d.ap().rearrange("(mt p) u -> p mt u", p=P)
    ho_v = ho_d.ap().rearrange("(mt p) u -> p mt u", p=P)
    co_v = co_d.ap().rearrange("(mt p) u -> p mt u", p=P)

    with tile.TileContext(nc) as tc, ExitStack() as ctx:
        consts = ctx.enter_context(tc.tile_pool(name="consts", bufs=1))
        ipool = ctx.enter_context(tc.tile_pool(name="ipool", bufs=MT))
        fpool = ctx.enter_context(tc.tile_pool(name="fpool", bufs=MT))
        thpool = ctx.enter_context(tc.tile_pool(name="thpool", bufs=MT))
        scratch = ctx.enter_context(tc.tile_pool(name="scratch", bufs=4))
        outp = ctx.enter_context(tc.tile_pool(name="outp", bufs=4))
        zpsum = ctx.enter_context(tc.tile_pool(name="zpsum", bufs=8, space="PSUM"))

        xh_sb = consts.tile([P, KG, MB], BF16)
        wr_sb = consts.tile([P, KG, NG], BF16)
        c_sb = consts.tile([P, MT, U], F32)

        # Warm-up operands: a zero tile the PE can multiply while the
        # first input slabs are still in flight (spends the slow pstate
        # window on junk work so real matmuls start near full clock).
        junk = consts.tile([P, P + NC], BF16)
        nc.gpsimd.memset(junk[:], 0.0)
        jpsum = zpsum.tile([P, NC], F32, tag="z", name="junkbank")
        for _ in range(5):
            nc.tensor.matmul(
                jpsum[:], junk[:, 0:P], junk[:, P : P + NC], start=True, stop=True
            )

        # --- DMA schedule.  All transfers have 1-2KB contiguous runs.
        # The DMA fabric is ~400GB/s but queues competing for it slow
        # each other down, so ALL inputs go FIFO down the single sync
        # queue in exact first-use order (a lone queue bursts ~350GB/s):
        # interleaved xh/Wi single-kg slabs pace the i-phase from ~9us,
        # then gate-f, c, gates g,o.  The other queues carry only the
        # eight h output DMAs, keeping their teardown drains short.
        for kg in range(KG):
            sl = slice(kg, kg + 1)
            nc.sync.dma_start(xh_sb[:, sl, :], xh_v[:, sl, :])
            nc.sync.dma_start(wr_sb[:, sl, 0:NC], wr_v[:, sl, 0:NC])
        nc.sync.dma_start(wr_sb[:, :, NC : 2 * NC], wr_v[:, :, NC : 2 * NC])
        nc.sync.dma_start(c_sb[:], c_v)
        for t in range(4):
            sl = slice(2 * t, 2 * t + 2)
            nc.sync.dma_start(wr_sb[:, sl, NG // 2 : NG], wr_v[:, sl, NG // 2 : NG])

        bias_bc = None
        if with_bias:
            assert b_d is not None
            bias_bc = consts.tile([P, NG], F32)
            b_ap = b_d.ap()
            nc.gpsimd.dma_start(
                out=bias_bc,
                in_=bass.AP(tensor=b_ap.tensor, offset=b_ap.offset, ap=[[0, P], [1, NG]]),
            )

        def add_bias(zp, n):
            if bias_bc is not None:
                nc.vector.tensor_add(zp[:], zp[:], bias_bc[:, n * NC : (n + 1) * NC])

        def z_part(n, mt, off, width, name):
            """Accumulate z[:, n*NC+off : +width] for m-tile mt (kg-inner)."""
            zp = zpsum.tile([P, width], F32, tag="z", name=name)
            for kg in range(KG):
                nc.tensor.matmul(
                    zp[:],
                    xh_sb[:, kg, mt * P : (mt + 1) * P],
                    wr_sb[:, kg, n * NC + off : n * NC + off + width],
                    start=(kg == 0),
                    stop=(kg == KG - 1),
                )
            if bias_bc is not None:
                nc.vector.tensor_add(
                    zp[:], zp[:], bias_bc[:, n * NC + off : n * NC + off + width]
                )
            return zp

        def z_chunk(n, mt):
            return z_part(n, mt, 0, NC, "zchunk")

        # --- i-phase: kg-outer / mt-inner across 8 PSUM banks, paced by
        # the xh kg-pair arrivals.
        i_banks = [
            zpsum.tile([P, NC], F32, tag="z", name=f"zbank{mt}") for mt in range(MT)
        ]
        for kg in range(KG):
            for mt in range(MT):
                nc.tensor.matmul(
                    i_banks[mt][:],
                    xh_sb[:, kg, mt * P : (mt + 1) * P],
                    wr_sb[:, kg, GI * NC : (GI + 1) * NC],
                    start=(kg == 0),
                    stop=(kg == KG - 1),
                )
        i_t = {}
        for mt in range(MT):
            add_bias(i_banks[mt], GI)
            it = ipool.tile([P, NC], F32, tag="i")
            nc.scalar.activation(it[:], i_banks[mt][:], SIG)
            i_t[mt] = it

        # --- f-phase: f = sigmoid(z1); fc = f*c_old precomputed here so
        # the per-tile tail chain in the fused phase is one op shorter.
        fc_t = {}
        for mt in range(MT):
            zf = z_chunk(GF, mt)
            ft = scratch.tile([P, NC], F32, tag="act")
            nc.scalar.activation(ft[:], zf[:], SIG)
            fc = fpool.tile([P, NC], F32, tag="fc")
            nc.vector.tensor_mul(fc[:], ft[:], c_sb[:, mt, :])
            fc_t[mt] = fc

        # --- fused g+o phase per m-tile: h/c results stream out as soon
        # as each tile completes, so the slow (~131GB/s/queue) output
        # path overlaps the remaining matmuls instead of trailing them.
        # h outputs ride the scalar queue (gpsimd stays DMA-free so its
        # teardown drain is short).
        for mt in range(MT - 1):
            zg = z_chunk(GG, mt)
            gt = scratch.tile([P, NC], F32, tag="act")
            nc.scalar.activation(gt[:], zg[:], TANH)
            c_new = outp.tile([P, NC], F32, tag="cnew")
            nc.vector.tensor_mul(c_new[:], i_t.pop(mt)[:], gt[:])
            nc.vector.tensor_add(c_new[:], c_new[:], fc_t.pop(mt)[:])
            nc.sync.dma_start(co_v[:, mt, :], c_new[:])
            th = thpool.tile([P, NC], F32, tag="th")
            nc.scalar.activation(th[:], c_new[:], TANH)

            zo = z_chunk(GO, mt)
            ot = scratch.tile([P, NC], F32, tag="act")
            nc.scalar.activation(ot[:], zo[:], SIG)
            h_new = outp.tile([P, NC], F32, tag="hnew")
            nc.vector.tensor_mul(h_new[:], ot[:], th[:])
            nc.scalar.dma_start(ho_v[:, mt, :], h_new[:])

        # Last m-tile runs in half-width chunks so the serial
        # sigmoid->tanh->mul->DMA tail after the final matmul is halved,
        # with the two h halves striped across the sync/scalar queues.
        mt = MT - 1
        H = NC // 2
        it7, fc7 = i_t.pop(mt), fc_t.pop(mt)
        c_new = outp.tile([P, NC], F32, tag="cnew")
        th7 = thpool.tile([P, NC], F32, tag="th")
        for hh in range(2):
            sl = slice(hh * H, (hh + 1) * H)
            zg = z_part(GG, mt, hh * H, H, "zg7")
            gt = scratch.tile([P, H], F32, tag="acth")
            nc.scalar.activation(gt[:], zg[:], TANH)
            nc.vector.tensor_mul(c_new[:, sl], it7[:, sl], gt[:])
            nc.vector.tensor_add(c_new[:, sl], c_new[:, sl], fc7[:, sl])
            nc.sync.dma_start(co_v[:, mt, sl], c_new[:, sl])
            nc.scalar.activation(th7[:, sl], c_new[:, sl], TANH)
        for hh in range(2):
            sl = slice(hh * H, (hh + 1) * H)
            zo = z_part(GO, mt, hh * H, H, "zo7")
            ot = scratch.tile([P, H], F32, tag="acth")
            nc.scalar.activation(ot[:], zo[:], SIG)
            h_new = outp.tile([P, H], F32, tag="hnewh")
            nc.vector.tensor_mul(h_new[:], ot[:], th7[:, sl])
            if hh == 0:
                nc.sync.dma_start(ho_v[:, mt, sl], h_new[:])
            else:
                nc.scalar.dma_start(ho_v[:, mt, sl], h_new[:])

    nc.compile()
    return nc


def _get_nc(with_bias: bool):
    if with_bias not in _NC_CACHE:
        _NC_CACHE[with_bias] = _build_lstm_nc(with_bias)
    return _NC_CACHE[with_bias]


def _prep(inputs, h_tm1, c_tm1, kernel, recurrent_kernel, bias):
    """Host-side bf16 cast/transpose + per-core input maps."""
    x = np.asarray(inputs, dtype=np.float32)
    h = np.asarray(h_tm1, dtype=np.float32)
    c = np.ascontiguousarray(np.asarray(c_tm1, dtype=np.float32))
    w = np.asarray(kernel, dtype=np.float32)
    r = np.asarray(recurrent_kernel, dtype=np.float32)
    b = np.asarray(bias, dtype=np.float32)

    # Stacked transposed activations [KK, B] in bf16.
    xh_t = np.empty((KK, B), dtype=BF16NP)
    xh_t[:D] = x.T
    xh_t[D:] = h.T

    # Stacked weights [KK, NG] in bf16.
    wr = np.empty((KK, NG), dtype=BF16NP)
    wr[:D] = w
    wr[D:] = r

    with_bias = bool(np.any(b))
    in_maps = []
    for core in range(N_CORES):
        sl = slice(core * MB, (core + 1) * MB)
        m = {
            "xh_t": np.ascontiguousarray(xh_t[:, sl]),
            "wr": wr,
            "c_tm1": np.ascontiguousarray(c[sl]),
        }
        if with_bias:
            m["bias"] = b
        in_maps.append(m)
    return in_maps, with_bias


def kernel(inputs, h_tm1, c_tm1, kernel, recurrent_kernel, bias):
    in_maps, with_bias = _prep(inputs, h_tm1, c_tm1, kernel, recurrent_kernel, bias)
    nc = _get_nc(with_bias)
    for _attempt in range(3):
        res = run_bass_kernel_spmd(nc, in_maps, core_ids=list(range(N_CORES)))
        h_out = np.concatenate([r_["h_out"] for r_ in res.results], axis=0)
        c_out = np.concatenate([r_["c_out"] for r_ in res.results], axis=0)
        # Very first execution after device bring-up has been seen to
        # return garbage when a previous session's teardown overlaps;
        # a clean rerun is cheap insurance.
        if np.isfinite(h_out).all() and np.isfinite(c_out).all():
            break
    return (h_out, h_out, c_out)


# revision 16
# speedup vs baseline: 1.0298x; 1.0298x over previous
"""Trainium2 Bass kernel for a custom LSTM cell step.

Reference computation (per full problem, B=8192, D=U=512):
    z = inputs @ kernel + h_tm1 @ recurrent_kernel + bias        # [B, 4U]
    i, f, g, o = split(z, 4, axis=1)
    i, f, o = sigmoid(...)  ;  g = tanh(g)
    c = f * c_tm1 + i * g
    h = o * tanh(c)
    return (h, h, c)

Sharding: data-parallel over the batch dim across 8 NeuronCores
(1024 rows per core); weights replicated.

Kernel strategy (final, ~72.7us vs 91.9us f32r baseline):
  - The PE is the bottleneck. Measured on hw: one [K=128]x[128,512]
    matmul streams at ~216ns regardless of f32r/bf16 (1 cyc/row at
    ~2.4GHz; DVFS sometimes caps it at ~259ns). fp8 DoubleRow doubles
    K per instruction but the 3-term hi/lo decomposition needed for
    precision (pure fp8 rel err 3.8e-2 > 2e-2 gate) costs 1.5x bf16's
    cycles — measured 117us — so bf16 is optimal: 256 matmuls ~= 55us.
  - x/h are cast to bf16 AND pre-transposed on the host into a stacked
    [x^T; h^T] tensor: no PE transposes, no PSUM->SBUF copies. W/R are
    host-stacked [W; R] in bf16 (halves all weight DMA).
    bf16 quantization end-to-end rel err ~2.4e-3 (gate is 2e-2).
  - DMA: concurrently-active queues contend for the ~400GB/s fabric
    and delay each other's completions, so ALL inputs go FIFO down the
    single sync queue in exact first-use order (xh/Wi single-kg slabs
    interleaved, then Wf, c, Wg+Wo); transfers keep 1-2KB contiguous
    runs. Five junk matmuls on a zeroed tile warm the PE out of its
    low-p-state while the first slabs land (~9us).
  - i-phase runs kg-outer/mt-inner across all 8 PSUM banks, paced by
    the arriving xh slabs; the f-phase (mt-outer) also precomputes
    fc = f*c_old; the fused g+o phase finishes each m-tile completely
    and streams c/h out immediately (outputs cap at ~131GB/s/queue, so
    they must overlap the matmul stream; h rides the scalar queue and
    gpsimd stays DMA-free for a short teardown drain).
  - The last m-tile runs in half-width chunks striped across the
    sync/scalar queues, halving the serial sigmoid->tanh->mul->DMA
    tail after the final matmul. Measured tail ~6us incl. ~2.8us fixed
    NEFF teardown; fixed preamble before the first DMA is ~7us.
"""

from contextlib import ExitStack

import ml_dtypes
import numpy as np

import concourse.bass as bass
import concourse.mybir as mybir
import concourse.tile as tile
from concourse import bacc
from concourse.bass_utils import run_bass_kernel_spmd

# Problem sizes (hardcoded per spec).
B, D, U = 8192, 512, 512
N_CORES = 8
MB = B // N_CORES  # 1024 batch rows per core
P = 128
MT = MB // P  # 8 m-tiles per core
KK = D + U  # 1024 stacked contraction dim (x|h vs W|R)
KG = KK // P  # 8 k-groups of 128
NG = 4 * U  # 2048 gate columns
NC = 512  # gate chunk width (one gate)

F32 = mybir.dt.float32
BF16 = mybir.dt.bfloat16
FP8 = mybir.dt.float8e4
BF16NP = ml_dtypes.bfloat16
E4NP = ml_dtypes.float8_e4m3
S8 = 16.0  # pre-scale for the fp8 f-gate operands
SINV = 1.0 / (S8 * S8)  # descale fused into sigmoid(f)
DR = mybir.MatmulPerfMode.DoubleRow

SIG = mybir.ActivationFunctionType.Sigmoid
TANH = mybir.ActivationFunctionType.Tanh

# Gate column chunks: 0=i, 1=f, 2=g, 3=o
GI, GF, GG, GO = 0, 1, 2, 3

_NC_CACHE: dict = {}


def _build_lstm_nc(with_bias: bool):
    """Build and compile the per-core Bass program."""
    nc = bacc.Bacc("TRN2", target_bir_lowering=False, debug=False)

    xh_d = nc.dram_tensor("xh_t", [KK, MB], BF16, kind="ExternalInput")
    wr_d = nc.dram_tensor("wr", [KK, NG], BF16, kind="ExternalInput")
    xh8_d = nc.dram_tensor("xh8_t", [KK, MB], FP8, kind="ExternalInput")
    wf8_d = nc.dram_tensor("wf8", [KK, NC], FP8, kind="ExternalInput")
    c_d = nc.dram_tensor("c_tm1", [MB, U], F32, kind="ExternalInput")
    b_d = None
    if with_bias:
        b_d = nc.dram_tensor("bias", [NG], F32, kind="ExternalInput")
    ho_d = nc.dram_tensor("h_out", [MB, U], F32, kind="ExternalOutput")
    co_d = nc.dram_tensor("c_out", [MB, U], F32, kind="ExternalOutput")

    # DRAM views tiled to [partition, group, free]
    xh_v = xh_d.ap().rearrange("(kg p) m -> p kg m", p=P)
    wr_v = wr_d.ap().rearrange("(kg p) n -> p kg n", p=P)
    xh8_v = xh8_d.ap().rearrange("(kg p) m -> p kg m", p=P)
    wf8_v = wf8_d.ap().rearrange("(kg p) n -> p kg n", p=P)
    c_v = c_d.ap().rearrange("(mt p) u -> p mt u", p=P)
    ho_v = ho_d.ap().rearrange("(mt p) u -> p mt u", p=P)
    co_v = co_d.ap().rearrange("(mt p) u -> p mt u", p=P)

    with tile.TileContext(nc) as tc, ExitStack() as ctx:
        consts = ctx.enter_context(tc.tile_pool(name="consts", bufs=1))
        ipool = ctx.enter_context(tc.tile_pool(name="ipool", bufs=MT))
        fpool = ctx.enter_context(tc.tile_pool(name="fpool", bufs=MT))
        thpool = ctx.enter_context(tc.tile_pool(name="thpool", bufs=MT))
        scratch = ctx.enter_context(tc.tile_pool(name="scratch", bufs=4))
        outp = ctx.enter_context(tc.tile_pool(name="outp", bufs=4))
        zpsum = ctx.enter_context(tc.tile_pool(name="zpsum", bufs=8, space="PSUM"))

        xh_sb = consts.tile([P, KG, MB], BF16)
        wr_sb = consts.tile([P, KG, NG], BF16)
        xh8_sb = consts.tile([P, KG, MB], FP8)
        wf8_sb = consts.tile([P, KG, NC], FP8)
        c_sb = consts.tile([P, MT, U], F32)

        # Warm-up operands: a zero tile the PE can multiply while the
        # first input slabs are still in flight (spends the slow pstate
        # window on junk work so real matmuls start near full clock).
        junk = consts.tile([P, P + NC], BF16)
        nc.gpsimd.memset(junk[:], 0.0)
        jpsum = zpsum.tile([P, NC], F32, tag="z", name="junkbank")
        for _ in range(5):
            nc.tensor.matmul(
                jpsum[:], junk[:, 0:P], junk[:, P : P + NC], start=True, stop=True
            )

        # --- DMA schedule.  All transfers have 1-2KB contiguous runs.
        # The DMA fabric is ~400GB/s but queues competing for it slow
        # each other down, so ALL inputs go FIFO down the single sync
        # queue in exact first-use order (a lone queue bursts ~350GB/s):
        # interleaved xh/Wi single-kg slabs pace the i-phase from ~9us,
        # then gate-f, c, gates g,o.  The other queues carry only the
        # eight h output DMAs, keeping their teardown drains short.
        for kg in range(KG):
            sl = slice(kg, kg + 1)
            nc.sync.dma_start(xh_sb[:, sl, :], xh_v[:, sl, :])
            nc.sync.dma_start(wr_sb[:, sl, 0:NC], wr_v[:, sl, 0:NC])
        nc.sync.dma_start(xh8_sb[:], xh8_v)
        nc.sync.dma_start(wf8_sb[:], wf8_v)
        nc.sync.dma_start(c_sb[:], c_v)
        for t in range(4):
            sl = slice(2 * t, 2 * t + 2)
            nc.sync.dma_start(wr_sb[:, sl, NG // 2 : NG], wr_v[:, sl, NG // 2 : NG])

        bias_bc = None
        bias256_bc = None
        if with_bias:
            assert b_d is not None
            bias_bc = consts.tile([P, NG], F32)
            b_ap = b_d.ap()
            nc.gpsimd.dma_start(
                out=bias_bc,
                in_=bass.AP(tensor=b_ap.tensor, offset=b_ap.offset, ap=[[0, P], [1, NG]]),
            )
            # the fp8 f-gate accumulates 256*z, so its bias must be
            # pre-scaled to match before the fused 1/256 descale
            bias256_bc = consts.tile([P, NG], F32)
            nc.scalar.activation(
                bias256_bc[:], bias_bc[:], mybir.ActivationFunctionType.Copy,
                scale=S8 * S8,
            )

        def add_bias(zp, n):
            if bias_bc is not None:
                nc.vector.tensor_add(zp[:], zp[:], bias_bc[:, n * NC : (n + 1) * NC])

        def z_part(n, mt, off, width, name):
            """Accumulate z[:, n*NC+off : +width] for m-tile mt (kg-inner)."""
            zp = zpsum.tile([P, width], F32, tag="z", name=name)
            for kg in range(KG):
                nc.tensor.matmul(
                    zp[:],
                    xh_sb[:, kg, mt * P : (mt + 1) * P],
                    wr_sb[:, kg, n * NC + off : n * NC + off + width],
                    start=(kg == 0),
                    stop=(kg == KG - 1),
                )
            if bias_bc is not None:
                nc.vector.tensor_add(
                    zp[:], zp[:], bias_bc[:, n * NC + off : n * NC + off + width]
                )
            return zp

        def z_chunk(n, mt):
            return z_part(n, mt, 0, NC, "zchunk")

        # --- i-phase: kg-outer / mt-inner across 8 PSUM banks, paced by
        # the xh kg-pair arrivals.
        i_banks = [
            zpsum.tile([P, NC], F32, tag="z", name=f"zbank{mt}") for mt in range(MT)
        ]
        for kg in range(KG):
            for mt in range(MT):
                nc.tensor.matmul(
                    i_banks[mt][:],
                    xh_sb[:, kg, mt * P : (mt + 1) * P],
                    wr_sb[:, kg, GI * NC : (GI + 1) * NC],
                    start=(kg == 0),
                    stop=(kg == KG - 1),
                )
        i_t = {}
        for mt in range(MT):
            add_bias(i_banks[mt], GI)
            it = ipool.tile([P, NC], F32, tag="i")
            nc.scalar.activation(it[:], i_banks[mt][:], SIG)
            i_t[mt] = it

        # --- f-phase: pure-fp8 DoubleRow matmuls (2048 cycles/tile vs
        # bf16's 4096; operands pre-scaled x16 on the host, descale 1/256
        # fused into the sigmoid).  The f gate tolerates fp8: measured
        # end-to-end rel err 1.34e-2 vs the 2e-2 gate.  fc = f*c_old is
        # precomputed here so the fused-phase tail chain is shorter.
        fc_t = {}
        for mt in range(MT):
            zf = zpsum.tile([P, NC], F32, tag="z", name="zf8")
            for t in range(4):
                sl = slice(2 * t, 2 * t + 2)
                nc.tensor.matmul(
                    zf[:],
                    xh8_sb[:, sl, mt * P : (mt + 1) * P],
                    wf8_sb[:, sl, :],
                    start=(t == 0),
                    stop=(t == 3),
                    perf_mode=DR,
                )
            if bias_bc is not None:
                nc.vector.tensor_add(zf[:], zf[:], bias256_bc[:, GF * NC : (GF + 1) * NC])
            ft = scratch.tile([P, NC], F32, tag="act")
            nc.scalar.activation(ft[:], zf[:], SIG, scale=SINV)
            fc = fpool.tile([P, NC], F32, tag="fc")
            nc.vector.tensor_mul(fc[:], ft[:], c_sb[:, mt, :])
            fc_t[mt] = fc

        # --- fused g+o phase per m-tile: h/c results stream out as soon
        # as each tile completes, so the slow (~131GB/s/queue) output
        # path overlaps the remaining matmuls instead of trailing them.
        # h outputs ride the scalar queue (gpsimd stays DMA-free so its
        # teardown drain is short).
        for mt in range(MT - 1):
            zg = z_chunk(GG, mt)
            gt = scratch.tile([P, NC], F32, tag="act")
            nc.scalar.activation(gt[:], zg[:], TANH)
            c_new = outp.tile([P, NC], F32, tag="cnew")
            nc.vector.tensor_mul(c_new[:], i_t.pop(mt)[:], gt[:])
            nc.vector.tensor_add(c_new[:], c_new[:], fc_t.pop(mt)[:])
            nc.sync.dma_start(co_v[:, mt, :], c_new[:])
            th = thpool.tile([P, NC], F32, tag="th")
            nc.scalar.activation(th[:], c_new[:], TANH)

            zo = z_chunk(GO, mt)
            ot = scratch.tile([P, NC], F32, tag="act")
            nc.scalar.activation(ot[:], zo[:], SIG)
            h_new = outp.tile([P, NC], F32, tag="hnew")
            nc.vector.tensor_mul(h_new[:], ot[:], th[:])
            nc.scalar.dma_start(ho_v[:, mt, :], h_new[:])

        # Last m-tile runs in half-width chunks so the serial
        # sigmoid->tanh->mul->DMA tail after the final matmul is halved,
        # with the two h halves striped across the sync/scalar queues.
        mt = MT - 1
        H = NC // 2
        it7, fc7 = i_t.pop(mt), fc_t.pop(mt)
        c_new = outp.tile([P, NC], F32, tag="cnew")
        th7 = thpool.tile([P, NC], F32, tag="th")
        for hh in range(2):
            sl = slice(hh * H, (hh + 1) * H)
            zg = z_part(GG, mt, hh * H, H, "zg7")
            gt = scratch.tile([P, H], F32, tag="acth")
            nc.scalar.activation(gt[:], zg[:], TANH)
            nc.vector.tensor_mul(c_new[:, sl], it7[:, sl], gt[:])
            nc.vector.tensor_add(c_new[:, sl], c_new[:, sl], fc7[:, sl])
            nc.sync.dma_start(co_v[:, mt, sl], c_new[:, sl])
            nc.scalar.activation(th7[:, sl], c_new[:, sl], TANH)
        for hh in range(2):
            sl = slice(hh * H, (hh + 1) * H)
            zo = z_part(GO, mt, hh * H, H, "zo7")
            ot = scratch.tile([P, H], F32, tag="acth")
            nc.scalar.activation(ot[:], zo[:], SIG)
            h_new = outp.tile([P, H], F32, tag="hnewh")
            nc.vector.tensor_mul(h_new[:], ot[:], th7[:, sl])
            if hh == 0:
                nc.sync.dma_start(ho_v[:, mt, sl], h_new[:])
            else:
                nc.scalar.dma_start(ho_v[:, mt, sl], h_new[:])

    nc.compile()
    return nc


def _get_nc(with_bias: bool):
    if with_bias not in _NC_CACHE:
        _NC_CACHE[with_bias] = _build_lstm_nc(with_bias)
    return _NC_CACHE[with_bias]


def _prep(inputs, h_tm1, c_tm1, kernel, recurrent_kernel, bias):
    """Host-side bf16 cast/transpose + per-core input maps."""
    x = np.asarray(inputs, dtype=np.float32)
    h = np.asarray(h_tm1, dtype=np.float32)
    c = np.ascontiguousarray(np.asarray(c_tm1, dtype=np.float32))
    w = np.asarray(kernel, dtype=np.float32)
    r = np.asarray(recurrent_kernel, dtype=np.float32)
    b = np.asarray(bias, dtype=np.float32)

    # Stacked transposed activations [KK, B] in bf16.
    xh_t = np.empty((KK, B), dtype=BF16NP)
    xh_t[:D] = x.T
    xh_t[D:] = h.T

    # Stacked weights [KK, NG] in bf16.
    wr = np.empty((KK, NG), dtype=BF16NP)
    wr[:D] = w
    wr[D:] = r

    # fp8 f-gate operands, pre-scaled x16 (quantized from fp32).
    xh8_t = np.empty((KK, B), dtype=E4NP)
    xh8_t[:D] = (x.T * S8).astype(E4NP)
    xh8_t[D:] = (h.T * S8).astype(E4NP)
    wf8 = np.empty((KK, NC), dtype=E4NP)
    wf8[:D] = (w[:, NC : 2 * NC] * S8).astype(E4NP)
    wf8[D:] = (r[:, NC : 2 * NC] * S8).astype(E4NP)

    with_bias = bool(np.any(b))
    in_maps = []
    for core in range(N_CORES):
        sl = slice(core * MB, (core + 1) * MB)
        m = {
            "xh_t": np.ascontiguousarray(xh_t[:, sl]),
            "wr": wr,
            "xh8_t": np.ascontiguousarray(xh8_t[:, sl]),
            "wf8": wf8,
            "c_tm1": np.ascontiguousarray(c[sl]),
        }
        if with_bias:
            m["bias"] = b
        in_maps.append(m)
    return in_maps, with_bias


def kernel(inputs, h_tm1, c_tm1, kernel, recurrent_kernel, bias):
    in_maps, with_bias = _prep(inputs, h_tm1, c_tm1, kernel, recurrent_kernel, bias)
    nc = _get_nc(with_bias)
    for _attempt in range(3):
        res = run_bass_kernel_spmd(nc, in_maps, core_ids=list(range(N_CORES)))
        h_out = np.concatenate([r_["h_out"] for r_ in res.results], axis=0)
        c_out = np.concatenate([r_["c_out"] for r_ in res.results], axis=0)
        # Very first execution after device bring-up has been seen to
        # return garbage when a previous session's teardown overlaps;
        # a clean rerun is cheap insurance.
        if np.isfinite(h_out).all() and np.isfinite(c_out).all():
            break
    return (h_out, h_out, c_out)


# revision 17
# speedup vs baseline: 1.0854x; 1.0541x over previous
"""Trainium2 Bass kernel for a custom LSTM cell step.

Reference computation (per full problem, B=8192, D=U=512):
    z = inputs @ kernel + h_tm1 @ recurrent_kernel + bias        # [B, 4U]
    i, f, g, o = split(z, 4, axis=1)
    i, f, o = sigmoid(...)  ;  g = tanh(g)
    c = f * c_tm1 + i * g
    h = o * tanh(c)
    return (h, h, c)

Sharding: data-parallel over the batch dim across 8 NeuronCores
(1024 rows per core); weights replicated.

Kernel strategy (final, ~72.7us vs 91.9us f32r baseline):
  - The PE is the bottleneck. Measured on hw: one [K=128]x[128,512]
    matmul streams at ~216ns regardless of f32r/bf16 (1 cyc/row at
    ~2.4GHz; DVFS sometimes caps it at ~259ns). fp8 DoubleRow doubles
    K per instruction but the 3-term hi/lo decomposition needed for
    precision (pure fp8 rel err 3.8e-2 > 2e-2 gate) costs 1.5x bf16's
    cycles — measured 117us — so bf16 is optimal: 256 matmuls ~= 55us.
  - x/h are cast to bf16 AND pre-transposed on the host into a stacked
    [x^T; h^T] tensor: no PE transposes, no PSUM->SBUF copies. W/R are
    host-stacked [W; R] in bf16 (halves all weight DMA).
    bf16 quantization end-to-end rel err ~2.4e-3 (gate is 2e-2).
  - DMA: concurrently-active queues contend for the ~400GB/s fabric
    and delay each other's completions, so ALL inputs go FIFO down the
    single sync queue in exact first-use order (xh/Wi single-kg slabs
    interleaved, then Wf, c, Wg+Wo); transfers keep 1-2KB contiguous
    runs. Five junk matmuls on a zeroed tile warm the PE out of its
    low-p-state while the first slabs land (~9us).
  - i-phase runs kg-outer/mt-inner across all 8 PSUM banks, paced by
    the arriving xh slabs; the f-phase (mt-outer) also precomputes
    fc = f*c_old; the fused g+o phase finishes each m-tile completely
    and streams c/h out immediately (outputs cap at ~131GB/s/queue, so
    they must overlap the matmul stream; h rides the scalar queue and
    gpsimd stays DMA-free for a short teardown drain).
  - The last m-tile runs in half-width chunks striped across the
    sync/scalar queues, halving the serial sigmoid->tanh->mul->DMA
    tail after the final matmul. Measured tail ~6us incl. ~2.8us fixed
    NEFF teardown; fixed preamble before the first DMA is ~7us.
"""

from contextlib import ExitStack

import ml_dtypes
import numpy as np

import concourse.bass as bass
import concourse.mybir as mybir
import concourse.tile as tile
from concourse import bacc
from concourse.bass_utils import run_bass_kernel_spmd

# Problem sizes (hardcoded per spec).
B, D, U = 8192, 512, 512
N_CORES = 8
MB = B // N_CORES  # 1024 batch rows per core
P = 128
MT = MB // P  # 8 m-tiles per core
KK = D + U  # 1024 stacked contraction dim (x|h vs W|R)
KG = KK // P  # 8 k-groups of 128
NG = 4 * U  # 2048 gate columns
NC = 512  # gate chunk width (one gate)

F32 = mybir.dt.float32
BF16 = mybir.dt.bfloat16
FP8 = mybir.dt.float8e4
BF16NP = ml_dtypes.bfloat16
E4NP = ml_dtypes.float8_e4m3
S8 = 16.0  # pre-scale for the fp8 f-gate operands
SINV = 1.0 / (S8 * S8)  # descale fused into sigmoid(f)
DR = mybir.MatmulPerfMode.DoubleRow

SIG = mybir.ActivationFunctionType.Sigmoid
TANH = mybir.ActivationFunctionType.Tanh

# Gate column chunks: 0=i, 1=f, 2=g, 3=o
GI, GF, GG, GO = 0, 1, 2, 3

_NC_CACHE: dict = {}


def _build_lstm_nc(with_bias: bool):
    """Build and compile the per-core Bass program."""
    nc = bacc.Bacc("TRN2", target_bir_lowering=False, debug=False)

    xh_d = nc.dram_tensor("xh_t", [KK, MB], BF16, kind="ExternalInput")
    wr_d = nc.dram_tensor("wr", [KK, NG], BF16, kind="ExternalInput")
    xh8_d = nc.dram_tensor("xh8_t", [KK, MB], FP8, kind="ExternalInput")
    wf8_d = nc.dram_tensor("wf8", [KK, NC], FP8, kind="ExternalInput")
    c_d = nc.dram_tensor("c_tm1", [MB, U], F32, kind="ExternalInput")
    b_d = None
    if with_bias:
        b_d = nc.dram_tensor("bias", [NG], F32, kind="ExternalInput")
    ho_d = nc.dram_tensor("h_out", [MB, U], F32, kind="ExternalOutput")
    co_d = nc.dram_tensor("c_out", [MB, U], F32, kind="ExternalOutput")

    # DRAM views tiled to [partition, group, free]
    xh_v = xh_d.ap().rearrange("(kg p) m -> p kg m", p=P)
    wr_v = wr_d.ap().rearrange("(kg p) n -> p kg n", p=P)
    xh8_v = xh8_d.ap().rearrange("(kg p) m -> p kg m", p=P)
    wf8_v = wf8_d.ap().rearrange("(kg p) n -> p kg n", p=P)
    c_v = c_d.ap().rearrange("(mt p) u -> p mt u", p=P)
    ho_v = ho_d.ap().rearrange("(mt p) u -> p mt u", p=P)
    co_v = co_d.ap().rearrange("(mt p) u -> p mt u", p=P)

    with tile.TileContext(nc) as tc, ExitStack() as ctx:
        consts = ctx.enter_context(tc.tile_pool(name="consts", bufs=1))
        ipool = ctx.enter_context(tc.tile_pool(name="ipool", bufs=MT))
        fpool = ctx.enter_context(tc.tile_pool(name="fpool", bufs=MT))
        thpool = ctx.enter_context(tc.tile_pool(name="thpool", bufs=MT))
        scratch = ctx.enter_context(tc.tile_pool(name="scratch", bufs=4))
        outp = ctx.enter_context(tc.tile_pool(name="outp", bufs=4))
        zpsum = ctx.enter_context(tc.tile_pool(name="zpsum", bufs=8, space="PSUM"))

        xh_sb = consts.tile([P, KG, MB], BF16)
        wr_sb = consts.tile([P, KG, NG], BF16)
        xh8_sb = consts.tile([P, KG, MB], FP8)
        wf8_sb = consts.tile([P, KG, NC], FP8)
        c_sb = consts.tile([P, MT, U], F32)

        # Warm-up operands: a zero tile the PE can multiply while the
        # first input slabs are still in flight (spends the slow pstate
        # window on junk work so real matmuls start near full clock).
        junk = consts.tile([P, P + NC], BF16)
        nc.gpsimd.memset(junk[:], 0.0)
        jpsum = zpsum.tile([P, NC], F32, tag="z", name="junkbank")
        for _ in range(5):
            nc.tensor.matmul(
                jpsum[:], junk[:, 0:P], junk[:, P : P + NC], start=True, stop=True
            )

        # --- DMA schedule.  All transfers have 1-2KB contiguous runs.
        # The DMA fabric is ~400GB/s but queues competing for it slow
        # each other down, so ALL inputs go FIFO down the single sync
        # queue in exact first-use order (a lone queue bursts ~350GB/s):
        # interleaved xh/Wi single-kg slabs pace the i-phase from ~9us,
        # then gate-f, c, gates g,o.  The other queues carry only the
        # eight h output DMAs, keeping their teardown drains short.
        for kg in range(KG):
            sl = slice(kg, kg + 1)
            nc.sync.dma_start(xh_sb[:, sl, :], xh_v[:, sl, :])
            nc.sync.dma_start(wr_sb[:, sl, 0:NC], wr_v[:, sl, 0:NC])
        nc.sync.dma_start(xh8_sb[:], xh8_v)
        nc.sync.dma_start(wf8_sb[:], wf8_v)
        # c arrives in m-tile-pair slabs threaded between the W halves:
        # fc(mt0-1) needs c early, but the late pairs are only consumed
        # by the fused phase tens of us later, so W gate-g must not wait
        # behind them (the fused phase is paced by the FIFO's tail).
        nc.sync.dma_start(c_sb[:, 0:2, :], c_v[:, 0:2, :])
        for t in range(4):
            sl = slice(2 * t, 2 * t + 2)
            nc.sync.dma_start(wr_sb[:, sl, 2 * NC : 3 * NC], wr_v[:, sl, 2 * NC : 3 * NC])
        nc.sync.dma_start(c_sb[:, 2:6, :], c_v[:, 2:6, :])
        for t in range(4):
            sl = slice(2 * t, 2 * t + 2)
            nc.sync.dma_start(wr_sb[:, sl, 3 * NC : NG], wr_v[:, sl, 3 * NC : NG])
        nc.sync.dma_start(c_sb[:, 6:8, :], c_v[:, 6:8, :])

        bias_bc = None
        bias256_bc = None
        if with_bias:
            assert b_d is not None
            bias_bc = consts.tile([P, NG], F32)
            b_ap = b_d.ap()
            nc.gpsimd.dma_start(
                out=bias_bc,
                in_=bass.AP(tensor=b_ap.tensor, offset=b_ap.offset, ap=[[0, P], [1, NG]]),
            )
            # the fp8 f-gate accumulates 256*z, so its bias must be
            # pre-scaled to match before the fused 1/256 descale
            bias256_bc = consts.tile([P, NG], F32)
            nc.scalar.activation(
                bias256_bc[:], bias_bc[:], mybir.ActivationFunctionType.Copy,
                scale=S8 * S8,
            )

        def add_bias(zp, n):
            if bias_bc is not None:
                nc.vector.tensor_add(zp[:], zp[:], bias_bc[:, n * NC : (n + 1) * NC])

        def z_part(n, mt, off, width, name):
            """Accumulate z[:, n*NC+off : +width] for m-tile mt (kg-inner)."""
            zp = zpsum.tile([P, width], F32, tag="z", name=name)
            for kg in range(KG):
                nc.tensor.matmul(
                    zp[:],
                    xh_sb[:, kg, mt * P : (mt + 1) * P],
                    wr_sb[:, kg, n * NC + off : n * NC + off + width],
                    start=(kg == 0),
                    stop=(kg == KG - 1),
                )
            if bias_bc is not None:
                nc.vector.tensor_add(
                    zp[:], zp[:], bias_bc[:, n * NC + off : n * NC + off + width]
                )
            return zp

        def z_chunk(n, mt):
            return z_part(n, mt, 0, NC, "zchunk")

        # --- i-phase: kg-outer / mt-inner across 8 PSUM banks, paced by
        # the xh kg-pair arrivals.
        i_banks = [
            zpsum.tile([P, NC], F32, tag="z", name=f"zbank{mt}") for mt in range(MT)
        ]
        for kg in range(KG):
            for mt in range(MT):
                nc.tensor.matmul(
                    i_banks[mt][:],
                    xh_sb[:, kg, mt * P : (mt + 1) * P],
                    wr_sb[:, kg, GI * NC : (GI + 1) * NC],
                    start=(kg == 0),
                    stop=(kg == KG - 1),
                )
        i_t = {}
        for mt in range(MT):
            add_bias(i_banks[mt], GI)
            it = ipool.tile([P, NC], F32, tag="i")
            nc.scalar.activation(it[:], i_banks[mt][:], SIG)
            i_t[mt] = it

        # --- f-phase: pure-fp8 DoubleRow matmuls (2048 cycles/tile vs
        # bf16's 4096; operands pre-scaled x16 on the host, descale 1/256
        # fused into the sigmoid).  The f gate tolerates fp8: measured
        # end-to-end rel err 1.34e-2 vs the 2e-2 gate.  fc = f*c_old is
        # precomputed here so the fused-phase tail chain is shorter.
        fc_t = {}
        for mt in range(MT):
            zf = zpsum.tile([P, NC], F32, tag="z", name="zf8")
            for t in range(4):
                sl = slice(2 * t, 2 * t + 2)
                nc.tensor.matmul(
                    zf[:],
                    xh8_sb[:, sl, mt * P : (mt + 1) * P],
                    wf8_sb[:, sl, :],
                    start=(t == 0),
                    stop=(t == 3),
                    perf_mode=DR,
                )
            if bias_bc is not None:
                nc.vector.tensor_add(zf[:], zf[:], bias256_bc[:, GF * NC : (GF + 1) * NC])
            ft = scratch.tile([P, NC], F32, tag="act")
            nc.scalar.activation(ft[:], zf[:], SIG, scale=SINV)
            fc = fpool.tile([P, NC], F32, tag="fc")
            nc.vector.tensor_mul(fc[:], ft[:], c_sb[:, mt, :])
            fc_t[mt] = fc

        # --- fused g+o phase per m-tile: h/c results stream out as soon
        # as each tile completes, so the slow (~131GB/s/queue) output
        # path overlaps the remaining matmuls instead of trailing them.
        # h outputs ride the scalar queue (gpsimd stays DMA-free so its
        # teardown drain is short).
        for mt in range(MT - 1):
            zg = z_chunk(GG, mt)
            gt = scratch.tile([P, NC], F32, tag="act")
            nc.scalar.activation(gt[:], zg[:], TANH)
            c_new = outp.tile([P, NC], F32, tag="cnew")
            nc.vector.tensor_mul(c_new[:], i_t.pop(mt)[:], gt[:])
            nc.vector.tensor_add(c_new[:], c_new[:], fc_t.pop(mt)[:])
            nc.sync.dma_start(co_v[:, mt, :], c_new[:])
            th = thpool.tile([P, NC], F32, tag="th")
            nc.scalar.activation(th[:], c_new[:], TANH)

            zo = z_chunk(GO, mt)
            ot = scratch.tile([P, NC], F32, tag="act")
            nc.scalar.activation(ot[:], zo[:], SIG)
            h_new = outp.tile([P, NC], F32, tag="hnew")
            nc.vector.tensor_mul(h_new[:], ot[:], th[:])
            nc.scalar.dma_start(ho_v[:, mt, :], h_new[:])

        # Last m-tile runs in half-width chunks so the serial
        # sigmoid->tanh->mul->DMA tail after the final matmul is halved,
        # with the two h halves striped across the sync/scalar queues.
        mt = MT - 1
        H = NC // 2
        it7, fc7 = i_t.pop(mt), fc_t.pop(mt)
        c_new = outp.tile([P, NC], F32, tag="cnew")
        th7 = thpool.tile([P, NC], F32, tag="th")
        for hh in range(2):
            sl = slice(hh * H, (hh + 1) * H)
            zg = z_part(GG, mt, hh * H, H, "zg7")
            gt = scratch.tile([P, H], F32, tag="acth")
            nc.scalar.activation(gt[:], zg[:], TANH)
            nc.vector.tensor_mul(c_new[:, sl], it7[:, sl], gt[:])
            nc.vector.tensor_add(c_new[:, sl], c_new[:, sl], fc7[:, sl])
            nc.sync.dma_start(co_v[:, mt, sl], c_new[:, sl])
            nc.scalar.activation(th7[:, sl], c_new[:, sl], TANH)
        for hh in range(2):
            sl = slice(hh * H, (hh + 1) * H)
            zo = z_part(GO, mt, hh * H, H, "zo7")
            ot = scratch.tile([P, H], F32, tag="acth")
            nc.scalar.activation(ot[:], zo[:], SIG)
            h_new = outp.tile([P, H], F32, tag="hnewh")
            nc.vector.tensor_mul(h_new[:], ot[:], th7[:, sl])
            if hh == 0:
                nc.sync.dma_start(ho_v[:, mt, sl], h_new[:])
            else:
                nc.scalar.dma_start(ho_v[:, mt, sl], h_new[:])

    nc.compile()
    return nc


def _get_nc(with_bias: bool):
    if with_bias not in _NC_CACHE:
        _NC_CACHE[with_bias] = _build_lstm_nc(with_bias)
    return _NC_CACHE[with_bias]


def _prep(inputs, h_tm1, c_tm1, kernel, recurrent_kernel, bias):
    """Host-side bf16 cast/transpose + per-core input maps."""
    x = np.asarray(inputs, dtype=np.float32)
    h = np.asarray(h_tm1, dtype=np.float32)
    c = np.ascontiguousarray(np.asarray(c_tm1, dtype=np.float32))
    w = np.asarray(kernel, dtype=np.float32)
    r = np.asarray(recurrent_kernel, dtype=np.float32)
    b = np.asarray(bias, dtype=np.float32)

    # Stacked transposed activations [KK, B] in bf16.
    xh_t = np.empty((KK, B), dtype=BF16NP)
    xh_t[:D] = x.T
    xh_t[D:] = h.T

    # Stacked weights [KK, NG] in bf16.
    wr = np.empty((KK, NG), dtype=BF16NP)
    wr[:D] = w
    wr[D:] = r

    # fp8 f-gate operands, pre-scaled x16 (quantized from fp32).
    xh8_t = np.empty((KK, B), dtype=E4NP)
    xh8_t[:D] = (x.T * S8).astype(E4NP)
    xh8_t[D:] = (h.T * S8).astype(E4NP)
    wf8 = np.empty((KK, NC), dtype=E4NP)
    wf8[:D] = (w[:, NC : 2 * NC] * S8).astype(E4NP)
    wf8[D:] = (r[:, NC : 2 * NC] * S8).astype(E4NP)

    with_bias = bool(np.any(b))
    in_maps = []
    for core in range(N_CORES):
        sl = slice(core * MB, (core + 1) * MB)
        m = {
            "xh_t": np.ascontiguousarray(xh_t[:, sl]),
            "wr": wr,
            "xh8_t": np.ascontiguousarray(xh8_t[:, sl]),
            "wf8": wf8,
            "c_tm1": np.ascontiguousarray(c[sl]),
        }
        if with_bias:
            m["bias"] = b
        in_maps.append(m)
    return in_maps, with_bias


def kernel(inputs, h_tm1, c_tm1, kernel, recurrent_kernel, bias):
    in_maps, with_bias = _prep(inputs, h_tm1, c_tm1, kernel, recurrent_kernel, bias)
    nc = _get_nc(with_bias)
    for _attempt in range(3):
        res = run_bass_kernel_spmd(nc, in_maps, core_ids=list(range(N_CORES)))
        h_out = np.concatenate([r_["h_out"] for r_ in res.results], axis=0)
        c_out = np.concatenate([r_["c_out"] for r_ in res.results], axis=0)
        # Very first execution after device bring-up has been seen to
        # return garbage when a previous session's teardown overlaps;
        # a clean rerun is cheap insurance.
        if np.isfinite(h_out).all() and np.isfinite(c_out).all():
            break
    return (h_out, h_out, c_out)
